# revision 20
# baseline (speedup 1.0000x reference)
"""MoE linear (modality-routed) Trainium2 kernel.

out[n] = x[n] @ W[modality_ids[n]].T + b[modality_ids[n]]

Strategy (data parallel over 8 cores, weight replicated):
- Host: per core shard of 16384 tokens, stable-argsort tokens by expert.
  Groups padded to a shared per-expert capacity (multiple of 128) so one
  SPMD NEFF serves all cores; per-tile expert is a compile-time constant.
- Device: input side uses batched dma_gather (one Pool instruction per
  G=8 128-token tiles, int16 indices wrap-16 across partitions) which
  amortizes the SWDGE fixed overhead; per tile: PE transpose
  (contraction dim to partitions) -> copy to SBUF on the Activation
  engine -> 4 accumulating fp32r matmuls against SBUF-resident W^T ->
  bias add on DVE (cast to bf16) -> per-tile indirect-DMA scatter of
  the bf16 row to the token's original row (host upcasts y to f32 while
  unsharding). The scatter's nominal out AP is a 128-row window;
  actual rows are selected by the index vector (base + idx*row_stride),
  padding slots point past bounds_check and are dropped.
"""

import sys

if "/opt/trn_rl_repo" not in sys.path:
    sys.path.insert(0, "/opt/trn_rl_repo")

import numpy as np
from ml_dtypes import bfloat16

import concourse.bass as bass  # noqa: F401
import concourse.tile as tile
from concourse import bacc, mybir
from concourse.bass import IndirectOffsetOnAxis
from concourse.bass_utils import run_bass_kernel_spmd
from concourse.masks import make_identity

N_CORES = 8
N_TOKENS = 131072
N_SHARD = N_TOKENS // N_CORES  # 16384
D_IN = 512
D_OUT = 512
N_EXPERTS = 3
P = 128
KC = D_IN // P  # 4 contraction chunks
G = 8  # tiles per dma_gather batch

_NC_CACHE = {}


def build_nc(n_shard, caps, num_devices=N_CORES):
    """Build + compile the SPMD Bass kernel for given per-expert capacities."""
    key = (n_shard, tuple(caps), num_devices)
    if key in _NC_CACHE:
        return _NC_CACHE[key]
    npad = sum(caps)
    nt = npad // P
    experts_of_tile = []
    for e, c in enumerate(caps):
        experts_of_tile += [e] * (c // P)

    nc = bacc.Bacc(
        "TRN2", target_bir_lowering=False, debug=False, num_devices=num_devices
    )
    f32 = mybir.dt.float32
    f32r = mybir.dt.float32r
    bf16 = mybir.dt.bfloat16
    i16 = mybir.dt.int16
    i32 = mybir.dt.int32

    # x is fed as bf16 (host-cast): halves gather traffic and speeds the PE
    # transposes; W/PSUM/bias/y stay f32.
    x = nc.dram_tensor("x", [n_shard, D_IN], bf16, kind="ExternalInput").ap()
    wt = nc.dram_tensor(
        "wt", [D_IN, N_EXPERTS * D_OUT], f32r, kind="ExternalInput"
    ).ap()
    bb = nc.dram_tensor(
        "bias_bc", [P, N_EXPERTS * D_OUT], f32, kind="ExternalInput"
    ).ap()
    gidx = nc.dram_tensor("gidx", [P, npad // 16], i16, kind="ExternalInput").ap()
    gdst = nc.dram_tensor("gdst", [P, nt], i32, kind="ExternalInput").ap()
    y = nc.dram_tensor("y", [n_shard, D_OUT], bf16, kind="ExternalOutput").ap()

    with tile.TileContext(nc) as tc:
        with (
            tc.tile_pool(name="const", bufs=1) as cpool,
            tc.tile_pool(name="xg", bufs=6) as xg_pool,
            tc.tile_pool(name="xt", bufs=4) as xt_pool,
            tc.tile_pool(name="outp", bufs=16) as out_pool,
            tc.tile_pool(name="ptr", bufs=4, space="PSUM") as ptr_pool,
            tc.tile_pool(name="pmm", bufs=4, space="PSUM") as pmm_pool,
        ):
            ident = cpool.tile([P, P], bf16)
            make_identity(nc, ident[:])
            # Hoisted registers: one RegisterMove instead of one per Pool DMA.
            bounds_reg = nc.gpsimd.to_reg(n_shard - 1)
            nidx_regs = {}
            for g in {G, nt % G or G}:
                nidx_regs[g] = nc.gpsimd.to_reg(g * P)

            # W^T resident in SBUF: block (e, kc) is [k=128, o=512]
            w_sb = cpool.tile([P, N_EXPERTS * KC * D_OUT], f32r)
            for e in range(N_EXPERTS):
                for kc in range(KC):
                    nc.sync.dma_start(
                        out=w_sb[:, (e * KC + kc) * D_OUT : (e * KC + kc + 1) * D_OUT],
                        in_=wt[kc * P : (kc + 1) * P, e * D_OUT : (e + 1) * D_OUT],
                    )
            bias_sb = cpool.tile([P, N_EXPERTS * D_OUT], f32)
            nc.sync.dma_start(out=bias_sb[:], in_=bb[:])
            gidx_sb = cpool.tile([P, npad // 16], i16)
            nc.sync.dma_start(out=gidx_sb[:], in_=gidx[:])
            gdst_sb = cpool.tile([P, nt], i32)
            nc.sync.dma_start(out=gdst_sb[:], in_=gdst[:])

            # Two software pipelines over program order:
            # - matmuls are emitted MM_DELAY tiles behind transposes, so the
            #   PE sequencer (head-of-line) never waits on the Act-engine
            #   PSUM->SBUF copy; the wait is hidden under later transposes.
            # - scatters are emitted SC_DELAY tiles behind the bias add, so
            #   the Pool sequencer never stalls on an unfinished tile.
            MM_DELAY = 2
            SC_DELAY = 10
            mm_pending = []
            sc_pending = []

            def emit_matmul(t, xt):
                e = experts_of_tile[t]
                pmm = pmm_pool.tile([P, D_OUT], f32)
                for kc in range(KC):
                    nc.tensor.matmul(
                        pmm[:],
                        lhsT=xt[:, kc * P : (kc + 1) * P],
                        rhs=w_sb[:, (e * KC + kc) * D_OUT : (e * KC + kc + 1) * D_OUT],
                        start=(kc == 0),
                        stop=(kc == KC - 1),
                    )
                osb = out_pool.tile([P, D_OUT], bf16)
                nc.vector.tensor_add(
                    out=osb[:],
                    in0=pmm[:],
                    in1=bias_sb[:, e * D_OUT : (e + 1) * D_OUT],
                )
                sc_pending.append((t, osb))
                if len(sc_pending) > SC_DELAY:
                    emit_scatter(*sc_pending.pop(0))

            def emit_scatter(t, osb):
                # Nominal out AP is a 128-row window; actual rows are
                # selected by gdst (base + idx*row_stride); padding rows
                # (idx = n_shard) exceed bounds_check and are dropped.
                # Scatters write disjoint row sets (the routing is 1:1), but
                # they share the same nominal window, which the Tile dep
                # tracker would chain as WAW — serializing every scatter at
                # full DMA latency. Cycle dep_tracking_offset over 16 fake
                # windows so only every 16th scatter chains.
                win = y[0:P]
                win = bass.AP(
                    tensor=win.tensor,
                    offset=win.offset,
                    ap=win.ap,
                    dep_tracking_offset=(t % 16) * P * D_OUT,
                )
                nc.gpsimd.indirect_dma_start(
                    out=win,
                    out_offset=IndirectOffsetOnAxis(ap=gdst_sb[:, t : t + 1], axis=0),
                    in_=osb[:],
                    in_offset=None,
                    bounds_check=bounds_reg,
                    oob_is_err=False,
                )

            for t0 in range(0, nt, G):
                g = min(G, nt - t0)
                # Batched gather: xg[p, j, :] = x[idxs[j*128+p]] where idxs
                # covers sorted slots [t0*128, (t0+g)*128).
                xg = xg_pool.tile([P, g, D_IN], bf16)
                nc.gpsimd.dma_gather(
                    xg[:],
                    x[:],
                    gidx_sb[:, t0 * (P // 16) : (t0 + g) * (P // 16)],
                    g * P,
                    nidx_regs[g],
                    D_IN,
                )
                for j in range(g):
                    t = t0 + j
                    ptr = ptr_pool.tile([P, D_IN], bf16)
                    for kc in range(KC):
                        nc.tensor.transpose(
                            ptr[:, kc * P : (kc + 1) * P],
                            xg[:, j, kc * P : (kc + 1) * P],
                            ident[:],
                        )
                    xt = xt_pool.tile([P, D_IN], f32r)
                    nc.scalar.copy(xt[:], ptr[:])
                    mm_pending.append((t, xt))
                    if len(mm_pending) > MM_DELAY:
                        emit_matmul(*mm_pending.pop(0))
            for t, xt in mm_pending:
                emit_matmul(t, xt)
            for t, osb in sc_pending:
                emit_scatter(t, osb)

    nc.compile()
    _NC_CACHE[key] = nc
    return nc


def make_routing(ids_shard, caps):
    """Per-core routing tables.

    gidx [P, npad//16] int16: dma_gather indices, wrap-16 per G-tile batch,
    replicated on 8x16 partitions. Padding slots gather row 0 (dropped later).
    gdst [P, nt] int32: per-tile scatter destinations; padding -> n_shard (OOB).
    """
    n_shard = ids_shard.shape[0]
    npad = sum(caps)
    nt = npad // P
    order = np.argsort(ids_shard, kind="stable").astype(np.int64)
    cnt = np.bincount(ids_shard, minlength=N_EXPERTS)
    gs = np.zeros(npad, np.int64)
    gd = np.full(npad, n_shard, np.int64)
    base = 0
    off = 0
    for e in range(N_EXPERTS):
        c = int(cnt[e])
        seg = order[off : off + c]
        gs[base : base + c] = seg
        gd[base : base + c] = seg
        base += caps[e]
        off += c
    blocks = []
    for t0 in range(0, nt, G):
        g = min(G, nt - t0)
        blk = gs[t0 * P : (t0 + g) * P]
        blocks.append(np.ascontiguousarray(blk.reshape(-1, 16).T))
    gidx = np.tile(np.concatenate(blocks, axis=1), (8, 1)).astype(np.int16)
    gdst = np.ascontiguousarray(gd.reshape(nt, P).T.astype(np.int32))
    return gidx, gdst


def prepare(inputs):
    """Shared host-side prep: returns (nc, in_maps)."""
    x = np.ascontiguousarray(np.asarray(inputs["x"], dtype=np.float32))
    ids = np.asarray(inputs["modality_ids"]).astype(np.int64)
    weight = np.asarray(inputs["weight"], dtype=np.float32)
    b = np.asarray(inputs["bias"], dtype=np.float32)

    wt = np.ascontiguousarray(weight.T)  # [D_IN, E*D_OUT]
    bias_bc = np.ascontiguousarray(
        np.broadcast_to(b[None, :], (P, N_EXPERTS * D_OUT))
    )

    counts = np.stack(
        [
            np.bincount(ids[c * N_SHARD : (c + 1) * N_SHARD], minlength=N_EXPERTS)
            for c in range(N_CORES)
        ]
    )
    caps = [int(-(-counts[:, e].max() // P) * P) for e in range(N_EXPERTS)]

    nc = build_nc(N_SHARD, caps)
    in_maps = []
    for c in range(N_CORES):
        ids_c = ids[c * N_SHARD : (c + 1) * N_SHARD]
        gidx, gdst = make_routing(ids_c, caps)
        in_maps.append(
            {
                "x": np.ascontiguousarray(
                    x[c * N_SHARD : (c + 1) * N_SHARD].astype(bfloat16)
                ),
                "wt": wt,
                "bias_bc": bias_bc,
                "gidx": gidx,
                "gdst": gdst,
            }
        )
    return nc, in_maps


def run(inputs, trace=False):
    """Returns (out, BassKernelResults)."""
    nc, in_maps = prepare(inputs)
    res = run_bass_kernel_spmd(nc, in_maps, list(range(N_CORES)), trace=trace)
    out = np.concatenate(
        [res.results[c]["y"] for c in range(N_CORES)], axis=0
    ).astype(np.float32)  # bf16 -> f32 upcast during unshard
    return out, res


def kernel(**inputs):
    out, _ = run(inputs, trace=False)
    return out


# revision 21
# speedup vs baseline: 1.0506x; 1.0506x over previous
"""MoE linear (modality-routed) Trainium2 kernel.

out[n] = x[n] @ W[modality_ids[n]].T + b[modality_ids[n]]

Strategy (data parallel over 8 cores, weight replicated):
- Host: per core shard of 16384 tokens, stable-argsort tokens by expert.
  Groups padded to a shared per-expert capacity (multiple of 128) so one
  SPMD NEFF serves all cores; per-tile expert is a compile-time constant.
- Device: input side uses batched dma_gather (one Pool instruction per
  G=8 128-token tiles, int16 indices wrap-16 across partitions) which
  amortizes the SWDGE fixed overhead; per tile: PE transpose
  (contraction dim to partitions) -> copy to SBUF on the Activation
  engine -> 4 accumulating fp32r matmuls against SBUF-resident W^T ->
  bias add on DVE (cast to bf16) -> per-tile indirect-DMA scatter of
  the bf16 row to the token's original row (host upcasts y to f32 while
  unsharding). The scatter's nominal out AP is a 128-row window;
  actual rows are selected by the index vector (base + idx*row_stride),
  padding slots point past bounds_check and are dropped.
"""

import sys

if "/opt/trn_rl_repo" not in sys.path:
    sys.path.insert(0, "/opt/trn_rl_repo")

import numpy as np
from ml_dtypes import bfloat16

import concourse.bass as bass  # noqa: F401
import concourse.tile as tile
from concourse import bacc, mybir
from concourse.bass import IndirectOffsetOnAxis
from concourse.bass_utils import run_bass_kernel_spmd
from concourse.masks import make_identity

N_CORES = 8
N_TOKENS = 131072
N_SHARD = N_TOKENS // N_CORES  # 16384
D_IN = 512
D_OUT = 512
N_EXPERTS = 3
P = 128
KC = D_IN // P  # 4 contraction chunks
G = 8  # tiles per dma_gather batch

_NC_CACHE = {}


def build_nc(n_shard, caps, num_devices=N_CORES):
    """Build + compile the SPMD Bass kernel for given per-expert capacities."""
    key = (n_shard, tuple(caps), num_devices)
    if key in _NC_CACHE:
        return _NC_CACHE[key]
    npad = sum(caps)
    nt = npad // P
    experts_of_tile = []
    for e, c in enumerate(caps):
        experts_of_tile += [e] * (c // P)

    nc = bacc.Bacc(
        "TRN2", target_bir_lowering=False, debug=False, num_devices=num_devices
    )
    f32 = mybir.dt.float32
    f32r = mybir.dt.float32r
    bf16 = mybir.dt.bfloat16
    i16 = mybir.dt.int16
    i32 = mybir.dt.int32

    # x is fed as bf16 (host-cast): halves gather traffic and speeds the PE
    # transposes; W/PSUM/bias/y stay f32.
    x = nc.dram_tensor("x", [n_shard, D_IN], bf16, kind="ExternalInput").ap()
    wt = nc.dram_tensor(
        "wt", [D_IN, N_EXPERTS * D_OUT], f32r, kind="ExternalInput"
    ).ap()
    bb = nc.dram_tensor(
        "bias_bc", [P, N_EXPERTS * D_OUT], f32, kind="ExternalInput"
    ).ap()
    gidx = nc.dram_tensor("gidx", [P, npad // 16], i16, kind="ExternalInput").ap()
    gdst = nc.dram_tensor("gdst", [P, nt], i32, kind="ExternalInput").ap()
    y = nc.dram_tensor("y", [n_shard, D_OUT], bf16, kind="ExternalOutput").ap()

    with tile.TileContext(nc) as tc:
        with (
            tc.tile_pool(name="const", bufs=1) as cpool,
            tc.tile_pool(name="xg", bufs=6) as xg_pool,
            tc.tile_pool(name="xt", bufs=4) as xt_pool,
            tc.tile_pool(name="outp", bufs=16) as out_pool,
            tc.tile_pool(name="ptr", bufs=4, space="PSUM") as ptr_pool,
            tc.tile_pool(name="pmm", bufs=4, space="PSUM") as pmm_pool,
        ):
            ident = cpool.tile([P, P], bf16)
            make_identity(nc, ident[:])
            # Hoisted registers: one RegisterMove instead of one per Pool DMA.
            bounds_reg = nc.gpsimd.to_reg(n_shard - 1)
            nidx_regs = {}
            for g in {G, nt % G or G}:
                nidx_regs[g] = nc.gpsimd.to_reg(g * P)

            # Routing tables first: they gate the first gather/scatter,
            # while the (larger) weight loads are only needed by the first
            # matmul, ~10us later.
            gidx_sb = cpool.tile([P, npad // 16], i16)
            nc.sync.dma_start(out=gidx_sb[:], in_=gidx[:])
            gdst_sb = cpool.tile([P, nt], i32)
            nc.sync.dma_start(out=gdst_sb[:], in_=gdst[:])
            bias_sb = cpool.tile([P, N_EXPERTS * D_OUT], f32)
            nc.sync.dma_start(out=bias_sb[:], in_=bb[:])
            # W^T resident in SBUF: block (e, kc) is [k=128, o=512]
            w_sb = cpool.tile([P, N_EXPERTS * KC * D_OUT], f32r)
            for e in range(N_EXPERTS):
                for kc in range(KC):
                    nc.sync.dma_start(
                        out=w_sb[:, (e * KC + kc) * D_OUT : (e * KC + kc + 1) * D_OUT],
                        in_=wt[kc * P : (kc + 1) * P, e * D_OUT : (e + 1) * D_OUT],
                    )

            # Two software pipelines over program order:
            # - matmuls are emitted MM_DELAY tiles behind transposes, so the
            #   PE sequencer (head-of-line) never waits on the Act-engine
            #   PSUM->SBUF copy; the wait is hidden under later transposes.
            # - scatters are emitted SC_DELAY tiles behind the bias add, so
            #   the Pool sequencer never stalls on an unfinished tile.
            MM_DELAY = 2
            SC_DELAY = 10
            mm_pending = []
            sc_pending = []

            def emit_matmul(t, xt):
                e = experts_of_tile[t]
                pmm = pmm_pool.tile([P, D_OUT], f32)
                for kc in range(KC):
                    nc.tensor.matmul(
                        pmm[:],
                        lhsT=xt[:, kc * P : (kc + 1) * P],
                        rhs=w_sb[:, (e * KC + kc) * D_OUT : (e * KC + kc + 1) * D_OUT],
                        start=(kc == 0),
                        stop=(kc == KC - 1),
                    )
                osb = out_pool.tile([P, D_OUT], bf16)
                nc.vector.tensor_add(
                    out=osb[:],
                    in0=pmm[:],
                    in1=bias_sb[:, e * D_OUT : (e + 1) * D_OUT],
                )
                sc_pending.append((t, osb))
                if len(sc_pending) > SC_DELAY:
                    emit_scatter(*sc_pending.pop(0))

            def emit_scatter(t, osb):
                # Nominal out AP is a 128-row window; actual rows are
                # selected by gdst (base + idx*row_stride); padding rows
                # (idx = n_shard) exceed bounds_check and are dropped.
                # Scatters write disjoint row sets (the routing is 1:1), but
                # they share the same nominal window, which the Tile dep
                # tracker would chain as WAW — serializing every scatter at
                # full DMA latency. Cycle dep_tracking_offset over 16 fake
                # windows so only every 16th scatter chains.
                win = y[0:P]
                win = bass.AP(
                    tensor=win.tensor,
                    offset=win.offset,
                    ap=win.ap,
                    dep_tracking_offset=(t % 16) * P * D_OUT,
                )
                nc.gpsimd.indirect_dma_start(
                    out=win,
                    out_offset=IndirectOffsetOnAxis(ap=gdst_sb[:, t : t + 1], axis=0),
                    in_=osb[:],
                    in_offset=None,
                    bounds_check=bounds_reg,
                    oob_is_err=False,
                )

            for t0 in range(0, nt, G):
                g = min(G, nt - t0)
                # Batched gather: xg[p, j, :] = x[idxs[j*128+p]] where idxs
                # covers sorted slots [t0*128, (t0+g)*128).
                xg = xg_pool.tile([P, g, D_IN], bf16)
                nc.gpsimd.dma_gather(
                    xg[:],
                    x[:],
                    gidx_sb[:, t0 * (P // 16) : (t0 + g) * (P // 16)],
                    g * P,
                    nidx_regs[g],
                    D_IN,
                )
                for j in range(g):
                    t = t0 + j
                    ptr = ptr_pool.tile([P, D_IN], bf16)
                    for kc in range(KC):
                        nc.tensor.transpose(
                            ptr[:, kc * P : (kc + 1) * P],
                            xg[:, j, kc * P : (kc + 1) * P],
                            ident[:],
                        )
                    xt = xt_pool.tile([P, D_IN], f32r)
                    nc.scalar.copy(xt[:], ptr[:])
                    mm_pending.append((t, xt))
                    if len(mm_pending) > MM_DELAY:
                        emit_matmul(*mm_pending.pop(0))
            for t, xt in mm_pending:
                emit_matmul(t, xt)
            for t, osb in sc_pending:
                emit_scatter(t, osb)

    nc.compile()
    _NC_CACHE[key] = nc
    return nc


def make_routing(ids_shard, caps):
    """Per-core routing tables.

    gidx [P, npad//16] int16: dma_gather indices, wrap-16 per G-tile batch,
    replicated on 8x16 partitions. Padding slots gather row 0 (dropped later).
    gdst [P, nt] int32: per-tile scatter destinations; padding -> n_shard (OOB).
    """
    n_shard = ids_shard.shape[0]
    npad = sum(caps)
    nt = npad // P
    order = np.argsort(ids_shard, kind="stable").astype(np.int64)
    cnt = np.bincount(ids_shard, minlength=N_EXPERTS)
    gs = np.zeros(npad, np.int64)
    gd = np.full(npad, n_shard, np.int64)
    base = 0
    off = 0
    for e in range(N_EXPERTS):
        c = int(cnt[e])
        seg = order[off : off + c]
        gs[base : base + c] = seg
        gd[base : base + c] = seg
        base += caps[e]
        off += c
    blocks = []
    for t0 in range(0, nt, G):
        g = min(G, nt - t0)
        blk = gs[t0 * P : (t0 + g) * P]
        blocks.append(np.ascontiguousarray(blk.reshape(-1, 16).T))
    gidx = np.tile(np.concatenate(blocks, axis=1), (8, 1)).astype(np.int16)
    gdst = np.ascontiguousarray(gd.reshape(nt, P).T.astype(np.int32))
    return gidx, gdst


def prepare(inputs):
    """Shared host-side prep: returns (nc, in_maps)."""
    x = np.ascontiguousarray(np.asarray(inputs["x"], dtype=np.float32))
    ids = np.asarray(inputs["modality_ids"]).astype(np.int64)
    weight = np.asarray(inputs["weight"], dtype=np.float32)
    b = np.asarray(inputs["bias"], dtype=np.float32)

    wt = np.ascontiguousarray(weight.T)  # [D_IN, E*D_OUT]
    bias_bc = np.ascontiguousarray(
        np.broadcast_to(b[None, :], (P, N_EXPERTS * D_OUT))
    )

    counts = np.stack(
        [
            np.bincount(ids[c * N_SHARD : (c + 1) * N_SHARD], minlength=N_EXPERTS)
            for c in range(N_CORES)
        ]
    )
    caps = [int(-(-counts[:, e].max() // P) * P) for e in range(N_EXPERTS)]

    nc = build_nc(N_SHARD, caps)
    in_maps = []
    for c in range(N_CORES):
        ids_c = ids[c * N_SHARD : (c + 1) * N_SHARD]
        gidx, gdst = make_routing(ids_c, caps)
        in_maps.append(
            {
                "x": np.ascontiguousarray(
                    x[c * N_SHARD : (c + 1) * N_SHARD].astype(bfloat16)
                ),
                "wt": wt,
                "bias_bc": bias_bc,
                "gidx": gidx,
                "gdst": gdst,
            }
        )
    return nc, in_maps


def run(inputs, trace=False):
    """Returns (out, BassKernelResults)."""
    nc, in_maps = prepare(inputs)
    res = run_bass_kernel_spmd(nc, in_maps, list(range(N_CORES)), trace=trace)
    out = np.concatenate(
        [res.results[c]["y"] for c in range(N_CORES)], axis=0
    ).astype(np.float32)  # bf16 -> f32 upcast during unshard
    return out, res


def kernel(**inputs):
    out, _ = run(inputs, trace=False)
    return out
